# revision 22
# baseline (speedup 1.0000x reference)
"""Fused attention block (QKV -> softmax attention -> out-proj -> SiLU gate)
for Trainium2, data-parallel over batch across 8 NeuronCores.

Reference computation (per batch element b, all fp32 in the oracle):
    gate = silu(x @ W_gate + b_gate)                  [T, D]
    qkv  = (x @ W_qkv + b_qkv) -> q,k,v per head      [T, H, 3*hd]
    att  = softmax(q k^T / sqrt(hd)) v                [T, H, hd]
    out  = (att.flat @ W_out + b_out) * gate          [T, D]

Shapes hardcoded: B=8, T=1024, D=1024, H=16, hd=64. 8 cores, one batch
element per core; weights replicated. No collectives needed.

On-chip layout is "transposed" (features on partitions, tokens on free dim):
  - host pre-transposes x -> xT and pre-tiles everything to [128, 8, ...]
  - scoresT[s,t] = (kT)^T-block @ qT with two heads row-packed in the
    128x128 PE array (K=64 each, tile_position auto from base partition);
    both heads' scores land in one 2-bank PSUM tile so a single merged
    [128,1024] exp evacuates them
  - softmax skips max-subtraction (scores ~ N(0,1); exp is safe) and gets
    the denominator for free from an all-ones 65th column in the AV matmul
  - normalization: per-pair batched reciprocal + gpsimd partition_broadcast
  - AV for pair j is emitted two pairs behind its scores so the in-order
    PE stream always has matmul work while ACT evacuates exps; gate
    projection chains are spread through the loop as further PE filler
    (evacuated raw via DVE; SiLU applied at the tail when ACT is idle)
  - Wq/Wk share one SBUF slot with the exp tiles (same pool tag): the
    slot is recycled once the q/k projections consumed it
"""

import sys

if "/opt/trn_rl_repo" not in sys.path:
    sys.path.insert(0, "/opt/trn_rl_repo")

import ml_dtypes
import numpy as np

import concourse.bass as bass
import concourse.mybir as mybir
import concourse.tile as tile
from concourse import bacc
from concourse.bass_utils import run_bass_kernel_spmd

B, T, D, H = 8, 1024, 1024, 16
HD = D // H  # 64
P = 128
KT = D // P  # 8 k-tiles over the contraction dim
NB = D // P  # 8 output-row blocks
TTS = 512  # token chunk (one PSUM bank of fp32)
NT = T // TTS  # 2
NPAIR = H // 2  # 8 head pairs
SB = T // P  # 8 key blocks

BF16 = mybir.dt.bfloat16
F32 = mybir.dt.float32

_NC_CACHE = {}


def _interleave(units, chains):
    """Merge two emitter lists, spreading `chains` evenly between `units`."""
    out = []
    nu, nc_ = len(units), len(chains)
    ci = 0
    for i, u in enumerate(units):
        out.append(u)
        want = (i + 1) * nc_ // nu
        while ci < want:
            out.append(chains[ci])
            ci += 1
    out.extend(chains[ci:])
    return out


def _build_nc(reps: int = 1):
    """Build the single-core Bass/Tile program (SPMD: same on all cores)."""
    nc = bacc.Bacc()

    xT_d = nc.declare_dram_parameter("xT", [P, KT, T], BF16, isOutput=False)
    wqk_d = nc.declare_dram_parameter("Wqk", [P, KT, 2 * D], BF16, isOutput=False)
    wv_d = nc.declare_dram_parameter("Wv", [P, KT, D], BF16, isOutput=False)
    wo_d = nc.declare_dram_parameter("Wo", [P, KT, D], BF16, isOutput=False)
    wg_d = nc.declare_dram_parameter("Wg", [P, KT, D], BF16, isOutput=False)
    b4_d = nc.declare_dram_parameter("b4", [P, 4, NB], F32, isOutput=False)
    out_d = nc.declare_dram_parameter("out", [P, NB, T], F32, isOutput=True)

    with tile.TileContext(nc) as tc:
        with (
            tc.tile_pool(name="const", bufs=1) as cpool,
            tc.tile_pool(name="big", bufs=2) as bigpool,
            tc.tile_pool(name="tmp", bufs=2) as tpool,
            tc.tile_pool(name="wvg", bufs=1) as wvgpool,
            tc.tile_pool(name="rec", bufs=1) as rpool,
            tc.tile_pool(name="rec1", bufs=1) as rpool1,
            tc.tile_pool(name="rec2", bufs=2) as rpool2,
            tc.tile_pool(name="psmm", bufs=4, space="PSUM") as psmm,
            tc.tile_pool(name="pssc", bufs=2, space="PSUM") as pssc,
        ):
            # ---- persistent SBUF residents -----------------------------
            xT = cpool.tile([P, KT, T], BF16, tag="xT")
            wv = wvgpool.tile([P, KT, D], BF16, tag="wvg", name="wv")
            wo = cpool.tile([P, KT, D], BF16, tag="wo")
            wg = cpool.tile([P, KT, D], BF16, tag="wg")
            b4 = cpool.tile([P, 4, NB], F32, tag="b4")
            bq, bk, bo, bg = b4[:, 0], b4[:, 1], b4[:, 2], b4[:, 3]
            # v with an interleaved all-ones 65th column per head: the AV
            # matmul's 65th output row is then the softmax denominator.
            v_sb = cpool.tile([P, KT, H * (HD + 1)], BF16, tag="v")
            qT = cpool.tile([P, NPAIR, T], BF16, tag="qT")
            kTt = cpool.tile([P, NPAIR, T], BF16, tag="kT")
            oT = cpool.tile([P, KT, T], BF16, tag="oT")
            gateT_cell = []  # pre-SiLU gate; lazily takes over wv's slot

            # Wqk shares the "big" tag with the 4MB exp tiles: its slot is
            # recycled for exp pair 1 once the q/k projections are done.
            wqk = bigpool.tile([P, KT, 2 * D], BF16, tag="big")

            nc.sync.dma_start(xT[:], xT_d[:])
            for j_ in range(NPAIR):
                for qk_ in (0, 1):
                    sl_ = slice(qk_ * D + j_ * P, qk_ * D + (j_ + 1) * P)
                    nc.sync.dma_start(wqk[:, :, sl_], wqk_d[:, :, sl_])
            nc.sync.dma_start(b4[:], b4_d[:])
            nc.sync.dma_start(wv[:], wv_d[:])
            nc.sync.dma_start(wg[:], wg_d[:])
            nc.sync.dma_start(wo[:], wo_d[:])

            v_heads = v_sb.rearrange("p s (h c) -> p s h c", c=HD + 1)
            nc.vector.memset(v_heads[:, :, :, HD : HD + 1], 1.0)

            # ---------------- emitter helpers ---------------------------
            def qk_chain(j, qk, tt):
                """q^T or k^T projection chain for pair j, token chunk tt."""

                def emit():
                    tsl = slice(tt * TTS, (tt + 1) * TTS)
                    dst, b_ = (qT, bq) if qk == 0 else (kTt, bk)
                    ps = psmm.tile([P, TTS], F32, tag="mm")
                    for kt in range(KT):
                        nc.tensor.matmul(
                            ps[:],
                            wqk[:, kt, qk * D + j * P : qk * D + (j + 1) * P],
                            xT[:, kt, tsl],
                            start=(kt == 0),
                            stop=(kt == KT - 1),
                        )
                    nc.vector.tensor_tensor(
                        dst[:, j, tsl],
                        ps[:],
                        b_[:, j : j + 1].to_broadcast((P, TTS)),
                        mybir.AluOpType.add,
                    )

                return emit

            def v_chain(tb, nn):
                def emit():
                    ps = psmm.tile([P, TTS], F32, tag="mm")
                    for kt in range(KT):
                        nc.tensor.matmul(
                            ps[:],
                            xT[:, kt, tb * P : (tb + 1) * P],
                            wv[:, kt, nn * TTS : (nn + 1) * TTS],
                            start=(kt == 0),
                            stop=(kt == KT - 1),
                        )
                    out_ap = v_heads[
                        :, tb, nn * (H // NT) : (nn + 1) * (H // NT), 0:HD
                    ]
                    ps_ap = ps[:].rearrange("p (h c) -> p h c", c=HD)
                    nc.vector.tensor_copy(out_ap, ps_ap)

                return emit

            def gate_chain(nb, tt):
                """Gate projection; evacuate RAW pre-SiLU via DVE (ACT is
                busy with exp during the loop; SiLU is applied at the tail)."""

                def emit():
                    if not gateT_cell:
                        gateT_cell.append(
                            wvgpool.tile([P, NB, T], BF16, tag="wvg", name="gateT")
                        )
                    gateT = gateT_cell[0]
                    tsl = slice(tt * TTS, (tt + 1) * TTS)
                    ps = psmm.tile([P, TTS], F32, tag="mm")
                    for kt in range(KT):
                        nc.tensor.matmul(
                            ps[:],
                            wg[:, kt, nb * P : (nb + 1) * P],
                            xT[:, kt, tsl],
                            start=(kt == 0),
                            stop=(kt == KT - 1),
                        )
                    nc.vector.tensor_tensor(
                        gateT[:, nb, tsl],
                        ps[:],
                        bg[:, nb : nb + 1].to_broadcast((P, TTS)),
                        mybir.AluOpType.add,
                    )

                return emit

            def sc_unit(j, e, u):
                """One scores+exp chunk: both heads of pair j, (sb, tt) = u."""

                def emit():
                    sb, tt = u // NT, u % NT
                    tsl = slice(tt * TTS, (tt + 1) * TTS)
                    ps2 = pssc.tile([P, 2 * TTS], F32, tag="sc")
                    nc.tensor.matmul(
                        ps2[:, 0:TTS],
                        kTt[0:HD, j, sb * P : (sb + 1) * P],
                        qT[0:HD, j, tsl],
                        start=True,
                        stop=True,
                    )
                    nc.tensor.matmul(
                        ps2[:, TTS : 2 * TTS],
                        kTt[HD:P, j, sb * P : (sb + 1) * P],
                        qT[HD:P, j, tsl],
                        start=True,
                        stop=True,
                    )
                    nc.scalar.activation(
                        e[:, sb, tt * 2 * TTS : (tt + 1) * 2 * TTS],
                        ps2[:],
                        mybir.ActivationFunctionType.Exp,
                        scale=float(HD**-0.5),
                    )

                return emit

            class AVPair:
                """AV chains for one pair + batched softmax normalization.

                e layout per sb: [tt=0: h1(512) h2(512), tt=1: h1 h2].
                """

                def __init__(self, j, e):
                    self.j, self.e = j, e
                    self.psu = {}
                    self.rb4 = None

                def chain(self, h_off, tt):
                    def emit():
                        j, e = self.j, self.e
                        h = 2 * j + h_off
                        psu = psmm.tile([P, TTS], F32, tag="mm", name="psu")
                        self.psu[(h_off, tt)] = psu
                        esl = slice(
                            tt * 2 * TTS + h_off * TTS,
                            tt * 2 * TTS + (h_off + 1) * TTS,
                        )
                        for sb in range(SB):
                            nc.tensor.matmul(
                                psu[0 : HD + 1, :],
                                v_sb[:, sb, h * (HD + 1) : (h + 1) * (HD + 1)],
                                e[:, sb, esl],
                                start=(sb == 0),
                                stop=(sb == SB - 1),
                            )
                        if self.rb4 is None:
                            self.rb4 = rpool.tile([P, TTS], F32, tag="rb4", name="rb4")
                        i = 32 * (h_off * NT + tt)
                        nc.vector.tensor_copy(
                            self.rb4[i : i + 1, :], psu[HD : HD + 1, :]
                        )
                        # evacuate unnormalized u now so the PSUM slot frees
                        # immediately; the oT slice is rescaled in place later
                        nc.vector.tensor_copy(
                            oT[
                                h_off * HD : (h_off + 1) * HD,
                                self.j,
                                tt * TTS : (tt + 1) * TTS,
                            ],
                            psu[0:HD, :],
                        )

                    return emit

                def normalize(self):
                    def emit():
                        # one batched reciprocal per pair: r rows sit at
                        # partitions {0,32,64,96}; partition_broadcast only
                        # works from a base-0 AP, so copy each row back down
                        rb4b = rpool.tile([P, TTS], F32, tag="rb4b", name="rb4b")
                        nc.vector.reciprocal(rb4b[0:97, :], self.rb4[0:97, :])
                        for (h_off, tt), psu in sorted(self.psu.items()):
                            i = 32 * (h_off * NT + tt)
                            tsl = slice(tt * TTS, (tt + 1) * TTS)
                            rbB = rpool2.tile([P, TTS], F32, tag="rbB")
                            if i:
                                nc.vector.tensor_copy(
                                    rbB[0:1, :], rb4b[i : i + 1, :]
                                )
                                nc.gpsimd.partition_broadcast(rbB[:], rbB[0:1, :])
                            else:
                                nc.gpsimd.partition_broadcast(rbB[:], rb4b[0:1, :])
                            o_ap = oT[h_off * HD : (h_off + 1) * HD, self.j, tsl]
                            nc.vector.tensor_tensor(
                                o_ap,
                                o_ap,
                                rbB[h_off * HD : (h_off + 1) * HD, :],
                                mybir.AluOpType.mult,
                            )

                    return emit

            def outproj_chain(nb, tt):
                def emit():
                    tsl = slice(tt * TTS, (tt + 1) * TTS)
                    ps = psmm.tile([P, TTS], F32, tag="mm")
                    for kt in range(KT):
                        nc.tensor.matmul(
                            ps[:],
                            wo[:, kt, nb * P : (nb + 1) * P],
                            oT[:, kt, tsl],
                            start=(kt == 0),
                            stop=(kt == KT - 1),
                        )
                    gtmp = tpool.tile([P, TTS], F32, tag="gtmp")
                    nc.scalar.activation(
                        gtmp[:],
                        gateT_cell[0][:, nb, tsl],
                        mybir.ActivationFunctionType.Silu,
                    )
                    yst = rpool1.tile([P, TTS], F32, tag="yst")
                    nc.vector.tensor_tensor(
                        yst[:],
                        ps[:],
                        bo[:, nb : nb + 1].to_broadcast((P, TTS)),
                        mybir.AluOpType.add,
                    )
                    nc.vector.tensor_tensor(
                        yst[:], yst[:], gtmp[:], mybir.AluOpType.mult
                    )
                    nc.sync.dma_start(out_d[:, nb, tsl], yst[:])

                return emit

            # ---------------- emission schedule -------------------------
            for rep in range(reps):
                e_tiles = {}

                def new_e(j):
                    e_tiles[j] = bigpool.tile([P, SB, 2 * D], BF16, tag="big", name=f"e{j}")
                    return e_tiles[j]

                gate_chains = [
                    gate_chain(nb, tt) for nb in range(NB) for tt in range(NT)
                ]

                # phase 1: q/k projections for pairs 0,1 (scores(0) deps)
                for em in [qk_chain(0, qk, tt) for qk in (0, 1) for tt in range(NT)]:
                    em()
                for em in [qk_chain(1, qk, tt) for qk in (0, 1) for tt in range(NT)]:
                    em()

                # phase 2: remaining q/k projections, scores(0) spread through
                e0 = new_e(0)
                units = [sc_unit(0, e0, u) for u in range(SB * NT)]
                chains = [
                    qk_chain(j, qk, tt)
                    for j in range(2, NPAIR)
                    for qk in (0, 1)
                    for tt in range(NT)
                ]
                for em in _interleave(units, chains):
                    em()

                # phase 3: v projections + scores(1) + AV(0) spread through
                avs = {0: AVPair(0, e_tiles[0])}
                e1 = new_e(1)
                units = [sc_unit(1, e1, u) for u in range(SB * NT)]
                chains = [v_chain(tb, nn) for tb in range(SB) for nn in range(NT)]
                chains += [
                    avs[0].chain(h_off, tt) for h_off in (0, 1) for tt in range(NT)
                ]
                chains.append(avs[0].normalize())
                for em in _interleave(units, chains):
                    em()

                # phase 4: steady loop; AV runs one pair behind its scores
                for j in range(2, NPAIR):
                    av = avs[j - 1] = AVPair(j - 1, e_tiles[j - 1])
                    ej = new_e(j)
                    units = [sc_unit(j, ej, u) for u in range(SB * NT)]
                    chains = [gate_chains.pop(0)]
                    chains += [
                        av.chain(h_off, tt) for h_off in (0, 1) for tt in range(NT)
                    ]
                    chains.append(av.normalize())
                    for em in _interleave(units, chains):
                        em()

                # tail: AV(7) interleaved with remaining gate chains as filler
                av = AVPair(NPAIR - 1, e_tiles[NPAIR - 1])
                avc = [av.chain(h_off, tt) for h_off in (0, 1) for tt in range(NT)]
                tail = _interleave(list(gate_chains), avc)
                gate_chains = []
                tail.append(av.normalize())
                for em in tail:
                    em()

                # out-proj + gate multiply
                for nb in range(NB):
                    for tt in range(NT):
                        outproj_chain(nb, tt)()

    nc.finalize()
    return nc


def _tile_pd(a):
    """[D, N] -> [128, D//128, N] (partition, k-tile, free), contiguous."""
    d, n = a.shape
    return np.ascontiguousarray(a.reshape(d // P, P, n).transpose(1, 0, 2))


def _to_bf16(a):
    return np.asarray(a, dtype=ml_dtypes.bfloat16)


def prep_in_maps(inputs):
    """Host-side prep: split/reorder weights, pre-transpose x, bf16-cast."""
    x = np.asarray(inputs["x"], np.float32)
    W_qkv = np.asarray(inputs["W_qkv"], np.float32)
    b_qkv = np.asarray(inputs["b_qkv"], np.float32)
    W_out = np.asarray(inputs["W_out"], np.float32)
    b_out = np.asarray(inputs["b_out"], np.float32)
    W_gate = np.asarray(inputs["W_gate"], np.float32)
    b_gate = np.asarray(inputs["b_gate"], np.float32)

    # split W_qkv into head-major W_q / W_k / W_v (head h owns qkv columns
    # [192h, 192h+192): q first 64, k next 64, v last 64)
    w3 = W_qkv.reshape(D, H, 3, HD)
    W_q = w3[:, :, 0, :].reshape(D, D)
    W_k = w3[:, :, 1, :].reshape(D, D)
    W_v = w3[:, :, 2, :].reshape(D, D)
    b3 = b_qkv.reshape(H, 3, HD)
    b_q = b3[:, 0, :].reshape(D)
    b_k = b3[:, 1, :].reshape(D)
    b_v = b3[:, 2, :].reshape(D)

    shared = {
        "Wqk": _to_bf16(np.concatenate([_tile_pd(W_q), _tile_pd(W_k)], axis=2)),
        "Wv": _to_bf16(_tile_pd(W_v)),
        "Wo": _to_bf16(_tile_pd(W_out)),
        "Wg": _to_bf16(_tile_pd(W_gate)),
        "b4": np.ascontiguousarray(
            np.stack(
                [
                    b_q.reshape(NB, P).T,
                    b_k.reshape(NB, P).T,
                    # b_v rides through attention untouched: fold into b_out
                    (b_out + b_v @ W_out).reshape(NB, P).T,
                    b_gate.reshape(NB, P).T,
                ],
                axis=1,
            )
        ),
    }
    in_maps = []
    for b in range(B):
        m = dict(shared)
        m["xT"] = _to_bf16(_tile_pd(np.ascontiguousarray(x[b].T)))
        in_maps.append(m)
    return in_maps


def kernel(x, W_qkv, b_qkv, W_out, b_out, W_gate, b_gate):
    in_maps = prep_in_maps(
        dict(x=x, W_qkv=W_qkv, b_qkv=b_qkv, W_out=W_out, b_out=b_out,
             W_gate=W_gate, b_gate=b_gate)
    )
    if "nc" not in _NC_CACHE:
        _NC_CACHE["nc"] = _build_nc()
    nc = _NC_CACHE["nc"]

    res = run_bass_kernel_spmd(nc, in_maps, list(range(B)))

    out = np.empty((B, T, D), np.float32)
    for b in range(B):
        yT = res.results[b]["out"].transpose(1, 0, 2).reshape(D, T)
        out[b] = yT.T
    return out


if __name__ == "__main__":
    rng = np.random.default_rng(0)
    ins = {
        "x": rng.standard_normal((B, T, D), dtype=np.float32),
        "W_qkv": (rng.standard_normal((D, 3 * D), dtype=np.float32) * D**-0.5),
        "b_qkv": np.zeros(3 * D, np.float32),
        "W_out": (rng.standard_normal((D, D), dtype=np.float32) * D**-0.5),
        "b_out": np.zeros(D, np.float32),
        "W_gate": (rng.standard_normal((D, D), dtype=np.float32) * D**-0.5),
        "b_gate": np.zeros(D, np.float32),
    }
    o = kernel(**ins)
    print("kernel ran, out shape", o.shape, "std", o.std())
